# revision 36
# baseline (speedup 1.0000x reference)
"""Bass/Tile kernel for the two-stage attention block (v3).

This environment (axon-tunneled trn2) executes engine instructions at
~30-90us each, serialized across engines, with DMA effectively free
(see microbench*.py). The design therefore minimizes weighted instruction
count rather than engine-seconds:

  - all matmuls f32r (bf16 matmuls/writes measured 2-6x slower here)
  - 736 matmuls total (floor at fp32 PSUM, N<=512/bank):
      96 proj1 + 128 scores1 + 128 apply1 + 96 proj2 + 128 scores2
      + 128 apply2 + 32 outproj
    scores-shape (K=64) costs ~27us, apply1-shape (64-col stationary) ~41us,
    proj-shape ~84us.
  - PSUM tiles are [128,1024] (2 banks) where possible; a single ACT
    activation(Identity, bias=...) evicts both banks in one ~33us op and
    folds the bias add.
  - P = sigmoid(S*mask) via: DVE mult (PSUM x mask -> slab slice) then ONE
    in-place ACT sigmoid over the whole [128, 8192] per-head slab.
  - stage-2 exp(S*scale+bias) folded into the 2-bank ACT eviction itself.
  - softmax denominator via 64 ones-columns appended per head in V2p:
    PSUM rows 64:128 of the apply2 output hold the replicated denominator.
  - everything on-chip stays transposed ([feature, token]); the final
    output is written as outT [DIM, N] and transposed on the host (free).

Layout chains (zero on-chip transposes):
  QT/KT = W.T @ x.T   : matmul(lhsT=W_chunk, rhs=xT_chunk)  -> [c, i]
  V     = x @ W       : matmul(lhsT=xT_chunk, rhs=Wv_chunk) -> [j, d]
  S^T   = (q@k.T).T   : matmul(lhsT=KT_h, rhs=QT_h)         -> [j, i]
  O^T   = (P@v).T     : matmul(lhsT=V_h, rhs=P^T_h)         -> [d, i]
  outT  = Wnn.T @ O2  : matmul(lhsT=Wnn_chunk, rhs=O2T)     -> [c, i]
"""

from contextlib import ExitStack

import concourse.bacc as bacc
import concourse.bass as bass
import concourse.tile as tile
from concourse import mybir
from concourse.vector_clock import ScopedClock

F32 = mybir.dt.float32
F32R = mybir.dt.float32r
AF = mybir.ActivationFunctionType
ALU = mybir.AluOpType

N, DIM, H, D = 1024, 512, 8, 64
SCALE = DIM**-0.5
KC = DIM // 128  # contraction chunks for projections
JC = N // 128  # key-side chunks (128 wide)
IC = N // 512  # query-side chunks (512 wide)
EXP_BIAS = -15.0
VP = 2 * D  # per-head width in padded V2: 64 data cols + 64 ones cols
_STOP_PHASE = 99


# ---------------------------------------------------------------------------
# Walrus in this container rejects instructions with >1 sync-wait.
# Split: hoist extra waits onto single-wait NoOps inserted just before.
def legalize_single_wait(nc):
    n_split = 0
    for fn in nc.m.functions:
        for blk in fn.blocks:
            insts = list(blk.instructions)
            out = []
            changed = False
            for inst in insts:
                si = inst.sync_info
                waits = list(si.on_wait) if (si is not None and si.on_wait) else []
                if len(waits) > 1:
                    changed = True
                    n_split += len(waits) - 1
                    for w in waits[:-1]:
                        nop = mybir.InstNoOp(
                            name=nc.get_next_instruction_name(),
                            sync_info=mybir.SyncInfo(on_wait=[w], on_update=[]),
                            bass_nofuse=True,
                            engine=inst.engine,
                        )
                        nc.register_instruction(nop)
                        out.append(nop)
                    si.on_wait = [waits[-1]]
                out.append(inst)
            if changed:
                blk.instructions = out
    return n_split


def _patched_drain_and_barrier(self, tick_clock, wait_clock):
    drain_inst = self.nc.sync.drain()
    wait_clock.add_sem_waits(
        drain_inst.ins, ScopedClock({None: tick_clock.global_clock})
    )
    si = drain_inst.ins.sync_info
    waits = list(si.on_wait or []) if si is not None else []
    if len(waits) > 1:
        si.on_wait = [waits[0]]
        for w in waits[1:]:
            extra = self.nc.sync.drain()
            esi = extra.ins.sync_info
            if esi is None:
                extra.ins.sync_info = mybir.SyncInfo(on_wait=[w], on_update=[])
            else:
                esi.on_wait = [w]

    self.nc.all_engine_barrier()
    assert self.sems is not None
    popped = self.nc._tile_sem_poison_stack.pop()
    assert popped is self._sem_poison
    self.nc.clear_and_free_semaphores(list(self.sems.allocated().values()))
    self.nc.all_engine_barrier()


def install_patches():
    tile.TileContext._drain_and_barrier = _patched_drain_and_barrier


# ---------------------------------------------------------------------------


def build_body(ctx: ExitStack, tc: tile.TileContext, d, out_ap, taps=None):
    nc = tc.nc

    def tap(name, ap):
        if taps is not None and name in taps:
            if ap.dtype != F32:
                ap = ap.bitcast(F32)
            nc.sync.dma_start(taps[name][:], ap)

    const_pool = ctx.enter_context(tc.tile_pool(name="const", bufs=1))

    def load_bias_cols(name, src, off):
        """[128,1] per c-chunk bias tiles (c on partitions)."""
        tiles = []
        for t in range(4):
            b = const_pool.tile([128, 1], F32, name=f"{name}_{t}")
            nc.sync.dma_start(b[:], src[off + t * 128 : off + (t + 1) * 128])
            tiles.append(b)
        return tiles

    bq1 = load_bias_cols("bq1", d["bqkv1"], 0)
    bk1 = load_bias_cols("bk1", d["bqkv1"], DIM)
    bq2 = load_bias_cols("bq2", d["bqkv2"], 0)
    bk2 = load_bias_cols("bk2", d["bqkv2"], DIM)
    bnn = load_bias_cols("bnn", d["bnn1"], 0)
    # host-prepared broadcast bias planes [128, DIM]
    bv1_b = const_pool.tile([128, DIM], F32, name="bv1b")
    nc.sync.dma_start(bv1_b[:], d["bv1b"][:, :])
    bv2_b = const_pool.tile([128, DIM], F32, name="bv2b")
    nc.sync.dma_start(bv2_b[:], d["bv2b"][:, :])

    expb = const_pool.tile([128, 1], F32, name="expb")
    nc.vector.memset(expb[:], EXP_BIAS)

    # --- tensors that span stage boundaries -------------------------------
    o1_pool = ctx.enter_context(tc.tile_pool(name="o1", bufs=1))
    O1T = [o1_pool.tile([128, N], F32R, name=f"O1T_{t}") for t in range(4)]

    s1 = ctx.enter_context(ExitStack())  # stage-1 scope: closed after phase 2
    qk1_pool = s1.enter_context(tc.tile_pool(name="qk1", bufs=1))
    QT1 = [qk1_pool.tile([128, N], F32R, name=f"QT1_{t}") for t in range(4)]
    KT1 = [qk1_pool.tile([128, N], F32R, name=f"KT1_{t}") for t in range(4)]
    V1 = [qk1_pool.tile([128, DIM], F32R, name=f"V1_{j}") for j in range(JC)]

    # mask slab: maskS[p, jc*N + i] = maskT[jc*128 + p, i]
    # (opened after the s1 pools; closed first, before s1, after phase 2)
    mask_scope = s1.enter_context(ExitStack())
    mask_sp = mask_scope.enter_context(tc.tile_pool(name="mask", bufs=1))
    maskS = mask_sp.tile([128, JC * N], F32, name="maskS")
    for jc in range(JC):
        nc.sync.dma_start(
            maskS[:, jc * N : (jc + 1) * N],
            d["maskT"][jc * 128 : (jc + 1) * 128, :],
        )

    # =====================================================================
    # Phase 1: stage-1 projections
    # =====================================================================
    def proj_qk(ps_pool, W, src, dst_tiles, biases, col0, pfx):
        """dst[c-chunk t] = W[:, col0+128t : col0+128(t+1)].T @ src + b,
        evicted per [128, 1024] 2-bank psum tile with bias folded in."""
        for t in range(4):
            ps = ps_pool.tile([128, N], F32, tag="proj_ps", name=f"{pfx}_{t}")
            for icx in range(IC):
                for kc in range(KC):
                    nc.tensor.matmul(
                        ps[:, icx * 512 : (icx + 1) * 512],
                        W[kc][:, col0 + t * 128 : col0 + (t + 1) * 128],
                        src[kc][:, icx * 512 : (icx + 1) * 512],
                        start=(kc == 0),
                        stop=(kc == KC - 1),
                    )
            nc.scalar.activation(dst_tiles[t][:], ps[:], AF.Identity,
                                 bias=biases[t][:])

    with tc.tile_pool(name="xw1", bufs=1) as xw1_pool, \
         tc.tile_pool(name="ps1", bufs=3, space="PSUM") as ps1_pool, \
         tc.tile_pool(name="psv1", bufs=2, space="PSUM") as psv1_pool:
        xT = [xw1_pool.tile([128, N], F32R, name=f"xT_{k}") for k in range(KC)]
        for k in range(KC):
            nc.sync.dma_start(xT[k][:], d["xT"][k * 128 : (k + 1) * 128, :])
        W1 = [xw1_pool.tile([128, 3 * DIM], F32R, name=f"W1_{k}")
              for k in range(KC)]
        for k in range(KC):
            nc.sync.dma_start(W1[k][:], d["Wqkv1"][k * 128 : (k + 1) * 128, :])

        proj_qk(ps1_pool, W1, xT, QT1, bq1, 0, "q1")
        proj_qk(ps1_pool, W1, xT, KT1, bk1, DIM, "k1")
        for j in range(JC):
            ps = psv1_pool.tile([128, DIM], F32, tag="v_ps", name=f"vps_{j}")
            for kc in range(KC):
                nc.tensor.matmul(
                    ps[:],
                    xT[kc][:, j * 128 : (j + 1) * 128],
                    W1[kc][:, 2 * DIM : 3 * DIM],
                    start=(kc == 0),
                    stop=(kc == KC - 1),
                )
            nc.vector.tensor_tensor(V1[j][:], ps[:], bv1_b[:], ALU.add)

        tap("qt1_0", QT1[0][:])
        tap("v1_0", V1[0][:])

    if _STOP_PHASE <= 1:
        raise StopIteration

    # =====================================================================
    # Phase 2: stage-1 attention (sigmoid(S * mask) @ V), transposed
    # Per head: 16 score MMs -> 8 DVE mask-mult evicts into a [128, 8192]
    # slab -> 1 in-place sigmoid -> 16 apply MMs -> shared-psum eviction.
    # =====================================================================
    with tc.tile_pool(name="p1", bufs=2) as p_pool, \
         tc.tile_pool(name="sps1", bufs=2, space="PSUM") as score_ps, \
         tc.tile_pool(name="aps1", bufs=2, space="PSUM") as apply_ps:
        for t in range(4):  # head pairs
            slabs = {}
            for h in (2 * t, 2 * t + 1):
                base = 64 * (h % 2)
                slab = p_pool.tile([128, JC * N], F32R, tag="p",
                                   name=f"P1_{h}")
                for jc in range(JC):
                    sps = score_ps.tile([128, N], F32, tag="score_ps",
                                        name=f"sps_{h}_{jc}")
                    for icx in range(IC):
                        nc.tensor.matmul(
                            sps[:, icx * 512 : (icx + 1) * 512],
                            KT1[t][base : base + 64, jc * 128 : (jc + 1) * 128],
                            QT1[t][base : base + 64, icx * 512 : (icx + 1) * 512],
                            start=True,
                            stop=True,
                        )
                    nc.scalar.copy(slab[:, jc * N : (jc + 1) * N], sps[:])
                # one cheap Pool-engine in-place multiply over the whole
                # [128, 8192] slab, then the in-place sigmoid
                nc.gpsimd.tensor_tensor(slab[:], slab[:], maskS[:], ALU.mult)
                nc.scalar.activation(slab[:], slab[:], AF.Sigmoid)
                slabs[h] = slab
            # apply per head into one [64, N] psum tile (icx halves side
            # by side) so the whole head evicts in a single ACT copy
            for h in (2 * t, 2 * t + 1):
                hb = 64 * (h % 2)
                aps = apply_ps.tile([64, N], F32, tag="apply_ps",
                                    name=f"aps1_{h}")
                for icx in range(IC):
                    for jc in range(JC):
                        nc.tensor.matmul(
                            aps[:, icx * 512 : (icx + 1) * 512],
                            V1[jc][:, h * D : (h + 1) * D],
                            slabs[h][:, jc * N + icx * 512 : jc * N + icx * 512 + 512],
                            start=(jc == 0),
                            stop=(jc == JC - 1),
                        )
                nc.scalar.copy(O1T[t][hb : hb + 64, :], aps[:])

    tap("o1t_0", O1T[0][:])

    if _STOP_PHASE <= 2:
        raise StopIteration
    s1.close()  # free QT1/KT1/V1 and the mask slab

    # =====================================================================
    # Phase 3: stage-2 projections (from O1T)
    # =====================================================================
    qk2_pool = ctx.enter_context(tc.tile_pool(name="qk2", bufs=1))
    QT2 = [qk2_pool.tile([128, N], F32R, name=f"QT2_{t}") for t in range(4)]
    KT2 = [qk2_pool.tile([128, N], F32R, name=f"KT2_{t}") for t in range(4)]
    V2p = [qk2_pool.tile([128, H * VP], F32R, name=f"V2p_{j}") for j in range(JC)]

    with tc.tile_pool(name="w2", bufs=1) as w2_pool, \
         tc.tile_pool(name="ps2", bufs=3, space="PSUM") as ps2_pool, \
         tc.tile_pool(name="psv2", bufs=2, space="PSUM") as psv2_pool:
        W2 = [w2_pool.tile([128, 3 * DIM], F32R, name=f"W2_{k}")
              for k in range(KC)]
        for k in range(KC):
            nc.sync.dma_start(W2[k][:], d["Wqkv2"][k * 128 : (k + 1) * 128, :])

        proj_qk(ps2_pool, W2, O1T, QT2, bq2, 0, "q2")
        proj_qk(ps2_pool, W2, O1T, KT2, bk2, DIM, "k2")
        for j in range(JC):
            ps = psv2_pool.tile([128, DIM], F32, tag="v2_ps", name=f"v2ps_{j}")
            for kc in range(KC):
                nc.tensor.matmul(
                    ps[:],
                    O1T[kc][:, j * 128 : (j + 1) * 128],
                    W2[kc][:, 2 * DIM : 3 * DIM],
                    start=(kc == 0),
                    stop=(kc == KC - 1),
                )
            # scatter per-head into the padded layout [j, h*128 + d]
            nc.vector.tensor_tensor(
                V2p[j][:, :].rearrange("p (h e) -> p h e", e=VP)[:, :, :D],
                ps[:].rearrange("p (h dd) -> p h dd", dd=D),
                bv2_b[:].rearrange("p (h dd) -> p h dd", dd=D),
                ALU.add,
            )
            # 64 ones columns per head (drives matmul-replicated denominators)
            nc.sync.dma_start(
                V2p[j][:, :].rearrange("p (h e) -> p h e", e=VP)[:, :, D:VP],
                d["onesb"][:, :].rearrange("p (h dd) -> p h dd", dd=D),
            )
        tap("qt2_0", QT2[0][:])
        tap("v2p_0", V2p[0][:])

    if _STOP_PHASE <= 3:
        raise StopIteration

    o2_pool = ctx.enter_context(tc.tile_pool(name="o2", bufs=1))
    O2T = [o2_pool.tile([128, N], F32R, name=f"O2T_{t}") for t in range(4)]

    # =====================================================================
    # Phase 4: stage-2 attention (softmax via exp + replicated denominators)
    # Per head: 16 score MMs -> 8 ACT exp evicts (scale+bias fused, 2-bank
    # reads) into slab -> 16 apply MMs (ones-padded V) -> recip+mult.
    # =====================================================================
    with tc.tile_pool(name="p2", bufs=2) as p2_pool, \
         tc.tile_pool(name="dscr", bufs=2) as d_pool, \
         tc.tile_pool(name="sps2", bufs=2, space="PSUM") as score2_ps, \
         tc.tile_pool(name="aps2", bufs=2, space="PSUM") as apply2_ps:
        for h in range(H):
            t, hb = h // 2, 64 * (h % 2)
            slab = p2_pool.tile([128, JC * N], F32R, tag="p2", name=f"P2_{h}")
            for jc in range(JC):
                sps = score2_ps.tile([128, N], F32, tag="score2_ps",
                                     name=f"s2ps_{h}_{jc}")
                for icx in range(IC):
                    nc.tensor.matmul(
                        sps[:, icx * 512 : (icx + 1) * 512],
                        KT2[t][hb : hb + 64, jc * 128 : (jc + 1) * 128],
                        QT2[t][hb : hb + 64, icx * 512 : (icx + 1) * 512],
                        start=True,
                        stop=True,
                    )
                nc.scalar.activation(
                    slab[:, jc * N : (jc + 1) * N], sps[:], AF.Exp,
                    bias=expb[:], scale=SCALE)
            aps = apply2_ps.tile([128, N], F32, tag="apply2_ps",
                                 name=f"aps2_{h}")
            for icx in range(IC):
                for jc in range(JC):
                    nc.tensor.matmul(
                        aps[:, icx * 512 : (icx + 1) * 512],
                        V2p[jc][:, h * VP : (h + 1) * VP],
                        slab[:, jc * N + icx * 512 : jc * N + icx * 512 + 512],
                        start=(jc == 0),
                        stop=(jc == JC - 1),
                    )
            # rows 0:64 unnormalized out, rows 64:128 replicated denominator
            db = d_pool.tile([64, N], F32, tag="db", name=f"db_{h}")
            nc.vector.reciprocal(db[:], aps[64:128, :])
            if h == 0:
                tap("db_00", db[:, 0:512])
            nc.vector.tensor_tensor(
                O2T[t][hb : hb + 64, :], aps[0:64, :], db[:], ALU.mult)

    tap("o2t_0", O2T[0][:])

    if _STOP_PHASE <= 4:
        raise StopIteration

    # =====================================================================
    # Phase 5: output projection -> outT [DIM, N] (host transposes)
    # =====================================================================
    with tc.tile_pool(name="wnn", bufs=1) as wnn_pool, \
         tc.tile_pool(name="outst", bufs=2) as out_pool, \
         tc.tile_pool(name="ps5", bufs=2, space="PSUM") as ps5_pool:
        Wnn = [wnn_pool.tile([128, DIM], F32R, name=f"Wnn_{k}")
               for k in range(KC)]
        for k in range(KC):
            nc.sync.dma_start(Wnn[k][:], d["Wnn1"][k * 128 : (k + 1) * 128, :])
        for t in range(4):  # output c-chunks
            ps = ps5_pool.tile([128, N], F32, tag="out_ps", name=f"ops_{t}")
            for icx in range(IC):
                for kc in range(KC):
                    nc.tensor.matmul(
                        ps[:, icx * 512 : (icx + 1) * 512],
                        Wnn[kc][:, t * 128 : (t + 1) * 128],
                        O2T[kc][:, icx * 512 : (icx + 1) * 512],
                        start=(kc == 0),
                        stop=(kc == KC - 1),
                    )
            ob = out_pool.tile([128, N], F32, tag="ob", name=f"ob_{t}")
            nc.scalar.activation(ob[:], ps[:], AF.Identity, bias=bnn[t][:])
            nc.sync.dma_start(out_ap[t * 128 : (t + 1) * 128, :], ob[:])


def build(n_repeat: int = 1, debug_taps: bool = False):
    install_patches()
    nc = bacc.Bacc("TRN2", target_bir_lowering=False, debug=False)
    d = {}

    def din(name, shape, dtype=F32):
        d[name] = nc.dram_tensor(name, shape, dtype, kind="ExternalInput").ap()

    din("xT", [DIM, N], F32R)
    din("maskT", [N, N])
    din("Wqkv1", [DIM, 3 * DIM], F32R)
    din("bqkv1", [3 * DIM])
    din("Wqkv2", [DIM, 3 * DIM], F32R)
    din("bqkv2", [3 * DIM])
    din("Wnn1", [DIM, DIM], F32R)
    din("bnn1", [DIM])
    din("bv1b", [128, DIM])
    din("bv2b", [128, DIM])
    din("onesb", [128, DIM], F32R)
    out_ap = nc.dram_tensor("outT", [DIM, N], F32, kind="ExternalOutput").ap()

    taps = None
    if debug_taps:
        shapes = {"qt1_0": [128, N], "v1_0": [128, DIM], "o1t_0": [128, N],
                  "qt2_0": [128, N], "v2p_0": [128, H * VP],
                  "db_00": [64, 512], "o2t_0": [128, N]}
        taps = {k: nc.dram_tensor(f"tap_{k}", v, F32, kind="ExternalOutput").ap()
                for k, v in shapes.items()}

    with tile.TileContext(nc) as tc:
        for _ in range(n_repeat):
            with ExitStack() as ctx:
                try:
                    build_body(ctx, tc, d, out_ap, taps=taps)
                except StopIteration:
                    pass

    nc.compile()
    n = legalize_single_wait(nc)
    return nc, n


# ===========================================================================
# Host-side entry point: full inputs in, full output out.
# Sharding: pure data-parallel — B=8 batch elements, one per NeuronCore.
# ===========================================================================
import numpy as np

_CACHED = {}


def _get_program():
    if "nc" not in _CACHED:
        _CACHED["nc"] = build(n_repeat=1)[0]
    return _CACHED["nc"]


def _make_common(mask, Wqkv1, bqkv1, Wqkv2, bqkv2, Wnn1, bnn1):
    f32 = lambda a: np.ascontiguousarray(np.asarray(a, dtype=np.float32))
    bqkv1, bqkv2, bnn1 = f32(bqkv1), f32(bqkv2), f32(bnn1)
    return {
        "maskT": f32(np.asarray(mask)[0, 0].T),
        "Wqkv1": f32(Wqkv1),
        "bqkv1": bqkv1,
        "Wqkv2": f32(Wqkv2),
        "bqkv2": bqkv2,
        "Wnn1": f32(Wnn1),
        "bnn1": bnn1,
        "bv1b": f32(np.broadcast_to(bqkv1[2 * DIM :], (128, DIM))),
        "bv2b": f32(np.broadcast_to(bqkv2[2 * DIM :], (128, DIM))),
        "onesb": np.ones((128, DIM), dtype=np.float32),
    }


def kernel(x, mask, Wqkv1, bqkv1, Wqkv2, bqkv2, Wnn1, bnn1):
    from concourse.bass_utils import run_bass_kernel_spmd

    x = np.asarray(x, dtype=np.float32)
    common = _make_common(mask, Wqkv1, bqkv1, Wqkv2, bqkv2, Wnn1, bnn1)
    in_maps = [
        {"xT": np.ascontiguousarray(x[c].T), **common} for c in range(x.shape[0])
    ]
    nc = _get_program()
    res = run_bass_kernel_spmd(nc, in_maps, core_ids=list(range(8)))
    return np.stack(
        [res.results[c]["outT"].T for c in range(8)]
    ).astype(np.float32)
